# revision 2
# baseline (speedup 1.0000x reference)
"""DeepSeekMoE Trainium2 kernel: 8-core expert-parallel sparse dispatch.

Strategy (hardcoded for D=5120, F=384, E=32, S=2, T=1024, top-2):
- Host computes the gate (softmax + top-2 + combine weights) and dispatches
  tokens: each of the 8 cores owns 4 routed experts; its assigned tokens are
  gathered, transposed and padded to 128 per expert on the host.
- Each core runs a Bass/Tile kernel: per expert a [tokens x D] @ [D x 2F]
  fc1 (tokens stationary, weights moving, fp32r), SwiGLU, PE transpose,
  fc2, and per-token combine scaling fused into the PSUM->SBUF copy.
- Shared experts are sharded expert x token-quarter (core c handles shared
  expert c%2 for tokens [256*(c//2), 256*(c//2+1))), so shared weights are
  not replicated across all cores.
- Host gathers: routed outputs are scatter-added via two vectorized gathers
  (each token has exactly 2 expert contributions), shared quarters are
  summed pairwise.
"""
import sys
import os

sys.path.insert(0, "/opt/trn_rl_repo")

import numpy as np

D = 5120
F = 384
F2 = 768
E = 32
S = 2
T = 1024
NCORE = 8
EPC = E // NCORE          # experts per core
CAP = 128                 # token capacity per expert slot
QT = T // (NCORE // S)    # tokens per shared quarter = 256
DT = D // 128             # 40 d-tiles
DTG = DT // 4             # 10 groups of 4 d-tiles
FT = F // 128             # 3 f-tiles

_compiled = {}


def _build(use_b1, use_b2, use_bs1):
    import concourse.bass as bass
    import concourse.bacc as bacc
    import concourse.tile as tile
    import concourse.mybir as mybir

    F32 = mybir.dt.float32
    F32R = mybir.dt.float32r
    AF = mybir.ActivationFunctionType

    nc = bacc.Bacc(None, target_bir_lowering=False)

    # ---- DRAM I/O ----
    # routed
    xg = nc.dram_tensor("xg", [EPC, DTG, 128, 4, CAP], F32R, kind="ExternalInput")
    w1p = nc.dram_tensor("w1p", [EPC, DTG, 128, 4, F2], F32R, kind="ExternalInput")
    w2p = nc.dram_tensor("w2p", [EPC, FT, 2, 128, 2560], F32R, kind="ExternalInput")
    cwc = nc.dram_tensor("cwc", [CAP, EPC], F32, kind="ExternalInput")
    out_r = nc.dram_tensor("out_r", [EPC, CAP, D], F32, kind="ExternalOutput")
    # shared (this core's expert s=c%2, token quarter q=c//2)
    xq = nc.dram_tensor("xq", [DTG, 128, 4, QT], F32R, kind="ExternalInput")
    w1sp = nc.dram_tensor("w1sp", [DTG, 128, 4, F2], F32R, kind="ExternalInput")
    w2sp = nc.dram_tensor("w2sp", [FT, 2, 128, 2560], F32R, kind="ExternalInput")
    out_s = nc.dram_tensor("out_s", [QT, D], F32, kind="ExternalOutput")
    # constants
    ident = nc.dram_tensor("ident", [128, 128], F32R, kind="ExternalInput")
    if use_b1:
        b1r = nc.dram_tensor("b1r", [EPC, F2], F32R, kind="ExternalInput")
    if use_b2:
        b2r = nc.dram_tensor("b2r", [EPC, D], F32R, kind="ExternalInput")
    if use_bs1:
        b1s = nc.dram_tensor("b1s", [1, F2], F32R, kind="ExternalInput")

    with tile.TileContext(nc) as tc:
        with (
            tc.tile_pool(name="cst", bufs=1) as cst,
            tc.tile_pool(name="wpool", bufs=3) as wpool,
            tc.tile_pool(name="w2pool", bufs=4) as w2pool,
            tc.tile_pool(name="xpool", bufs=3) as xpool,
            tc.tile_pool(name="spool", bufs=2) as spool,
            tc.tile_pool(name="opool", bufs=2) as opool,
            tc.tile_pool(name="ph", bufs=2, space="PSUM") as ph_pool,
            tc.tile_pool(name="pt", bufs=2, space="PSUM") as pt_pool,
            tc.tile_pool(name="po", bufs=2, space="PSUM") as po_pool,
        ):
            ident_t = cst.tile([128, 128], F32R)
            nc.gpsimd.dma_start(ident_t[:], ident[:])
            cw_t = cst.tile([CAP, EPC], F32)
            nc.gpsimd.dma_start(cw_t[:], cwc[:])
            need_ones = use_b1 or use_b2 or use_bs1
            if need_ones:
                ones_t = cst.tile([1, 128], F32R)
                nc.gpsimd.memset(ones_t[:], 1.0)
            if use_b1:
                b1r_t = cst.tile([EPC, F2], F32R)
                nc.gpsimd.dma_start(b1r_t[:], b1r[:])
            if use_b2:
                b2r_t = cst.tile([EPC, D], F32R)
                nc.gpsimd.dma_start(b2r_t[:], b2r[:])
            if use_bs1:
                b1s_t = cst.tile([1, F2], F32R)
                nc.gpsimd.dma_start(b1s_t[:], b1s[:])

            def swiglu_transpose(psum_h, tagsuffix):
                """psum_h [128,768] -> actT [128,3,128] (f-major, transposed)."""
                silu_t = spool.tile([128, F], F32, tag="silu")
                nc.scalar.activation(silu_t[:], psum_h[:, 0:F], AF.Silu)
                act_t = spool.tile([128, F], F32R, tag="act")
                nc.vector.tensor_mul(act_t[:], silu_t[:], psum_h[:, F:F2])
                actT = spool.tile([128, FT, 128], F32R, tag="actT" + tagsuffix)
                for ft in range(FT):
                    ptile = pt_pool.tile([128, 128], F32R)
                    nc.tensor.transpose(
                        ptile[:], act_t[:, ft * 128:(ft + 1) * 128], ident_t[:]
                    )
                    nc.vector.tensor_copy(actT[:, ft, :], ptile[:])
                return actT

            # ================= routed experts =================
            for e in range(EPC):
                psum_h = ph_pool.tile([128, F2], F32, tag="ph")
                if use_b1:
                    nc.tensor.matmul(psum_h[:, 0:512], ones_t[:, 0:CAP],
                                     b1r_t[e:e + 1, 0:512], start=True, stop=False)
                    nc.tensor.matmul(psum_h[:, 512:F2], ones_t[:, 0:CAP],
                                     b1r_t[e:e + 1, 512:F2], start=True, stop=False)
                for dtg in range(DTG):
                    xg_t = xpool.tile([128, 4, CAP], F32R, tag="xg")
                    nc.gpsimd.dma_start(xg_t[:], xg[e, dtg])
                    w1_t = wpool.tile([128, 4, F2], F32R, tag="w1")
                    nc.sync.dma_start(w1_t[:], w1p[e, dtg])
                    first = (dtg == 0) and not use_b1
                    for g in range(4):
                        nc.tensor.matmul(psum_h[:, 0:512], xg_t[:, g, :],
                                         w1_t[:, g, 0:512],
                                         start=first and g == 0, stop=False)
                        nc.tensor.matmul(psum_h[:, 512:F2], xg_t[:, g, :],
                                         w1_t[:, g, 512:F2],
                                         start=first and g == 0,
                                         stop=(dtg == DTG - 1 and g == 3))
                actT = swiglu_transpose(psum_h, "")
                ob = opool.tile([128, D], F32, tag="ob")
                for half in range(2):
                    w2_t = [None] * FT
                    for ft in range(FT):
                        w2_t[ft] = w2pool.tile([128, 2560], F32R, tag="w2")
                        nc.sync.dma_start(w2_t[ft][:], w2p[e, ft, half])
                    for ch in range(5):
                        po = po_pool.tile([128, 512], F32)
                        if use_b2:
                            nc.tensor.matmul(
                                po[:], ones_t[:, 0:CAP],
                                b2r_t[e:e + 1,
                                      half * 2560 + ch * 512:
                                      half * 2560 + (ch + 1) * 512],
                                start=True, stop=False)
                        for ft in range(FT):
                            nc.tensor.matmul(
                                po[:], actT[:, ft, :],
                                w2_t[ft][:, ch * 512:(ch + 1) * 512],
                                start=(ft == 0) and not use_b2,
                                stop=(ft == FT - 1))
                        nc.scalar.activation(
                            ob[:, half * 2560 + ch * 512:half * 2560 + (ch + 1) * 512],
                            po[:], AF.Copy, scale=cw_t[:, e:e + 1])
                nc.gpsimd.dma_start(out_r[e], ob[:])

            # ================= shared expert (this core's slice) ============
            psum_s = [ph_pool.tile([128, F2], F32, tag="ph") for _ in range(2)]
            if use_bs1:
                for tt in range(2):
                    nc.tensor.matmul(psum_s[tt][:, 0:512], ones_t[:, 0:128],
                                     b1s_t[:, 0:512], start=True, stop=False)
                    nc.tensor.matmul(psum_s[tt][:, 512:F2], ones_t[:, 0:128],
                                     b1s_t[:, 512:F2], start=True, stop=False)
            for dtg in range(DTG):
                xq_t = xpool.tile([128, 4, QT], F32R, tag="xq")
                nc.gpsimd.dma_start(xq_t[:], xq[dtg])
                w1s_t = wpool.tile([128, 4, F2], F32R, tag="w1")
                nc.sync.dma_start(w1s_t[:], w1sp[dtg])
                first = (dtg == 0) and not use_bs1
                for g in range(4):
                    for tt in range(2):
                        nc.tensor.matmul(
                            psum_s[tt][:, 0:512],
                            xq_t[:, g, tt * 128:(tt + 1) * 128],
                            w1s_t[:, g, 0:512],
                            start=first and g == 0, stop=False)
                        nc.tensor.matmul(
                            psum_s[tt][:, 512:F2],
                            xq_t[:, g, tt * 128:(tt + 1) * 128],
                            w1s_t[:, g, 512:F2],
                            start=first and g == 0,
                            stop=(dtg == DTG - 1 and g == 3))
            actTs = [swiglu_transpose(psum_s[tt], "s") for tt in range(2)]
            obs = [opool.tile([128, D], F32, tag="ob") for _ in range(2)]
            for half in range(2):
                w2s_t = [None] * FT
                for ft in range(FT):
                    w2s_t[ft] = w2pool.tile([128, 2560], F32R, tag="w2")
                    nc.sync.dma_start(w2s_t[ft][:], w2sp[ft, half])
                for tt in range(2):
                    for ch in range(5):
                        po = po_pool.tile([128, 512], F32)
                        for ft in range(FT):
                            nc.tensor.matmul(
                                po[:], actTs[tt][:, ft, :],
                                w2s_t[ft][:, ch * 512:(ch + 1) * 512],
                                start=(ft == 0), stop=(ft == FT - 1))
                        nc.vector.tensor_copy(
                            obs[tt][:, half * 2560 + ch * 512:
                                    half * 2560 + (ch + 1) * 512], po[:])
            for tt in range(2):
                nc.gpsimd.dma_start(out_s[tt * 128:(tt + 1) * 128, :], obs[tt][:])

    nc.compile()
    return nc


def _get_nc(key):
    if key not in _compiled:
        _compiled[key] = _build(*key)
    return _compiled[key]


def _silu(v):
    return v / (1.0 + np.exp(-v))


def _pack_w1(w):  # [D, 2F] -> [DTG, 128, 4, 2F]
    return np.ascontiguousarray(
        w.reshape(DTG, 4, 128, F2).transpose(0, 2, 1, 3))


def _pack_w2(w):  # [F, D] -> [FT, 2, 128, 2560]
    return np.ascontiguousarray(
        w.reshape(FT, 128, 2, 2560).transpose(0, 2, 1, 3))


def _pack_xT(xt_cols):  # [D, ncols] -> [DTG, 128, 4, ncols]
    n = xt_cols.shape[1]
    return np.ascontiguousarray(
        xt_cols.reshape(DTG, 4, 128, n).transpose(0, 2, 1, 3))


def kernel(x, gate_w, gate_b, shared_w1, shared_b1, shared_w2, shared_b2,
           routed_w1, routed_b1, routed_w2, routed_b2):
    from concourse.bass_utils import run_bass_kernel_spmd

    f32 = np.float32
    x = np.asarray(x, f32)
    gate_w = np.asarray(gate_w, f32)
    gate_b = np.asarray(gate_b, f32)
    shared_w1 = np.asarray(shared_w1, f32)
    shared_b1 = np.asarray(shared_b1, f32)
    shared_w2 = np.asarray(shared_w2, f32)
    shared_b2 = np.asarray(shared_b2, f32)
    routed_w1 = np.asarray(routed_w1, f32)
    routed_b1 = np.asarray(routed_b1, f32)
    routed_w2 = np.asarray(routed_w2, f32)
    routed_b2 = np.asarray(routed_b2, f32)

    B = x.shape[0]
    x2 = x.reshape(T, D)

    # ---- gate: softmax + top-2 (unnormalized combine weights) ----
    logits = x2 @ gate_w + gate_b
    m = logits.max(-1, keepdims=True)
    p = np.exp(logits - m, dtype=f32)
    p = p / p.sum(-1, keepdims=True)
    ar = np.arange(T)
    i1 = np.argmax(p, -1)
    p1 = p[ar, i1]
    pm = p.copy()
    pm[ar, i1] = -1.0
    i2 = np.argmax(pm, -1)
    p2 = p[ar, i2]

    # per-expert token lists (stable order)
    pairs = np.concatenate([i1, i2])
    toks = np.concatenate([ar, ar])
    wts = np.concatenate([p1, p2]).astype(f32)
    order = np.argsort(pairs, kind="stable")
    pairs_s, toks_s, wts_s = pairs[order], toks[order], wts[order]
    counts = np.bincount(pairs, minlength=E)
    starts = np.zeros(E + 1, np.int64)
    np.cumsum(counts, out=starts[1:])

    sel_tok = [None] * E
    sel_wt = [None] * E
    overflow = []
    for e in range(E):
        te = toks_s[starts[e]:starts[e + 1]]
        we = wts_s[starts[e]:starts[e + 1]]
        if len(te) > CAP:
            overflow.append((e, te[CAP:], we[CAP:]))
            te, we = te[:CAP], we[:CAP]
        sel_tok[e] = te
        sel_wt[e] = we

    use_b1 = bool(np.any(routed_b1))
    use_b2 = bool(np.any(routed_b2))
    use_bs1 = bool(np.any(shared_b1))
    nc = _get_nc((use_b1, use_b2, use_bs1))

    ident_np = np.eye(128, dtype=f32)
    xT = np.ascontiguousarray(x2.T)  # [D, T]

    in_maps = []
    for c in range(NCORE):
        es = [4 * c + i for i in range(EPC)]
        # gathered-padded tokens, one 128-slot per expert: [D, 4*CAP]
        idx_pad = np.zeros(EPC * CAP, np.int64)
        cw_pad = np.zeros((CAP, EPC), f32)
        for i, e in enumerate(es):
            n = len(sel_tok[e])
            idx_pad[i * CAP:i * CAP + n] = sel_tok[e]
            cw_pad[:n, i] = sel_wt[e]
        xg_cols = xT[:, idx_pad]  # [D, 512]
        xg_np = np.stack([
            _pack_xT(xg_cols[:, i * CAP:(i + 1) * CAP]) for i in range(EPC)])
        w1p_np = np.stack([_pack_w1(routed_w1[e]) for e in es])
        w2p_np = np.stack([_pack_w2(routed_w2[e]) for e in es])

        s_c, q_c = c % S, c // S
        xq_np = _pack_xT(xT[:, q_c * QT:(q_c + 1) * QT])
        w1sp_np = _pack_w1(shared_w1[s_c])
        w2sp_np = _pack_w2(shared_w2[s_c])

        im = {
            "xg": xg_np, "w1p": w1p_np, "w2p": w2p_np, "cwc": cw_pad,
            "xq": xq_np, "w1sp": w1sp_np, "w2sp": w2sp_np, "ident": ident_np,
        }
        if use_b1:
            im["b1r"] = np.ascontiguousarray(routed_b1[es])
        if use_b2:
            im["b2r"] = np.ascontiguousarray(routed_b2[es])
        if use_bs1:
            im["b1s"] = shared_b1[s_c:s_c + 1]
        in_maps.append(im)

    res = run_bass_kernel_spmd(nc, in_maps, core_ids=list(range(NCORE)))

    # ---- host gather/unshard ----
    # routed: each valid (expert, slot) row is c_t * expert_out(token)
    R = np.concatenate([res.results[c]["out_r"] for c in range(NCORE)], axis=0)
    R = R.reshape(E * CAP, D)
    tok_of_row = np.full(E * CAP, -1, np.int64)
    valid = np.zeros(E * CAP, bool)
    for e in range(E):
        n = len(sel_tok[e])
        tok_of_row[e * CAP:e * CAP + n] = sel_tok[e]
        valid[e * CAP:e * CAP + n] = True
    vrows = np.flatnonzero(valid)
    tv = tok_of_row[vrows]
    o = np.argsort(tv, kind="stable")
    out = np.zeros((T, D), f32)
    n_entries = np.bincount(tv, minlength=T)
    if n_entries.max() <= 2 and not overflow and n_entries.min() == 2:
        rows_sorted = vrows[o]
        out += R[rows_sorted[0::2]]
        out += R[rows_sorted[1::2]]
    else:
        np.add.at(out, tv, R[vrows])
    # overflow tokens: exact host fallback
    for e, te, we in overflow:
        xv = x2[te]
        h = xv @ routed_w1[e] + routed_b1[e]
        act = _silu(h[:, :F]) * h[:, F:]
        out[te] += we[:, None] * (act @ routed_w2[e] + routed_b2[e])

    # shared: quarters q handled by cores 2q (expert 0) and 2q+1 (expert 1)
    for q in range(NCORE // S):
        out[q * QT:(q + 1) * QT] += res.results[S * q]["out_s"]
        out[q * QT:(q + 1) * QT] += res.results[S * q + 1]["out_s"]
    out += shared_b2.sum(0)[None, :]

    return out.reshape(B, T, D).astype(f32)


# revision 3
# speedup vs baseline: 1.5400x; 1.5400x over previous
"""DeepSeekMoE Trainium2 kernel: 8-core expert-parallel sparse dispatch.

Strategy (hardcoded for D=5120, F=384, E=32, S=2, T=1024, top-2):
- Host computes the gate (softmax + top-2 + combine weights) and dispatches
  tokens: each of the 8 cores owns 4 routed experts; its assigned tokens are
  gathered, transposed and padded to 128 per expert on the host.
- Each core runs a Bass/Tile kernel: per expert a [tokens x D] @ [D x 2F]
  fc1 (tokens stationary, weights moving, fp32r), SwiGLU, PE transpose,
  fc2, and per-token combine scaling fused into the PSUM->SBUF copy.
- Shared experts are sharded expert x token-quarter (core c handles shared
  expert c%2 for tokens [256*(c//2), 256*(c//2+1))), so shared weights are
  not replicated across all cores.
- Host gathers: routed outputs are scatter-added via two vectorized gathers
  (each token has exactly 2 expert contributions), shared quarters are
  summed pairwise.
"""
import sys
import os

sys.path.insert(0, "/opt/trn_rl_repo")

import numpy as np

D = 5120
F = 384
F2 = 768
E = 32
S = 2
T = 1024
NCORE = 8
EPC = E // NCORE          # experts per core
CAP = 128                 # token capacity per expert slot
QT = T // (NCORE // S)    # tokens per shared quarter = 256
DT = D // 128             # 40 d-tiles
DTG = DT // 4             # 10 groups of 4 d-tiles
FT = F // 128             # 3 f-tiles

KDT = "bf16"   # matmul compute dtype: "bf16" or "f32r"
_compiled = {}


def _np_kdt():
    import concourse.mybir as mybir
    return mybir.dt.np(mybir.dt.bfloat16 if KDT == "bf16" else mybir.dt.float32r)


def _build(use_b1, use_b2, use_bs1, kdt):
    import concourse.bass as bass
    import concourse.bacc as bacc
    import concourse.tile as tile
    import concourse.mybir as mybir

    F32 = mybir.dt.float32
    F32R = mybir.dt.bfloat16 if kdt == "bf16" else mybir.dt.float32r
    AF = mybir.ActivationFunctionType

    nc = bacc.Bacc(None, target_bir_lowering=False)

    # ---- DRAM I/O ----
    # routed
    xg = nc.dram_tensor("xg", [EPC, DTG, 128, 4, CAP], F32R, kind="ExternalInput")
    w1p = nc.dram_tensor("w1p", [EPC, DTG, 128, 4, F2], F32R, kind="ExternalInput")
    w2p = nc.dram_tensor("w2p", [EPC, FT, 2, 128, 2560], F32R, kind="ExternalInput")
    cwc = nc.dram_tensor("cwc", [CAP, EPC], F32, kind="ExternalInput")
    out_r = nc.dram_tensor("out_r", [EPC, CAP, D], F32, kind="ExternalOutput")
    # shared (this core's expert s=c%2, token quarter q=c//2)
    xq = nc.dram_tensor("xq", [DTG, 128, 4, QT], F32R, kind="ExternalInput")
    w1sp = nc.dram_tensor("w1sp", [DTG, 128, 4, F2], F32R, kind="ExternalInput")
    w2sp = nc.dram_tensor("w2sp", [FT, 2, 128, 2560], F32R, kind="ExternalInput")
    out_s = nc.dram_tensor("out_s", [QT, D], F32, kind="ExternalOutput")
    # constants
    ident = nc.dram_tensor("ident", [128, 128], F32R, kind="ExternalInput")
    if use_b1:
        b1r = nc.dram_tensor("b1r", [EPC, F2], F32R, kind="ExternalInput")
    if use_b2:
        b2r = nc.dram_tensor("b2r", [EPC, D], F32R, kind="ExternalInput")
    if use_bs1:
        b1s = nc.dram_tensor("b1s", [1, F2], F32R, kind="ExternalInput")

    with tile.TileContext(nc) as tc:
        with (
            tc.tile_pool(name="cst", bufs=1) as cst,
            tc.tile_pool(name="wpool", bufs=3) as wpool,
            tc.tile_pool(name="w2pool", bufs=4) as w2pool,
            tc.tile_pool(name="xpool", bufs=3) as xpool,
            tc.tile_pool(name="spool", bufs=2) as spool,
            tc.tile_pool(name="opool", bufs=2) as opool,
            tc.tile_pool(name="ph", bufs=2, space="PSUM") as ph_pool,
            tc.tile_pool(name="pt", bufs=2, space="PSUM") as pt_pool,
            tc.tile_pool(name="po", bufs=2, space="PSUM") as po_pool,
        ):
            ident_t = cst.tile([128, 128], F32R)
            nc.gpsimd.dma_start(ident_t[:], ident[:])
            cw_t = cst.tile([CAP, EPC], F32)
            nc.gpsimd.dma_start(cw_t[:], cwc[:])
            need_ones = use_b1 or use_b2 or use_bs1
            if need_ones:
                ones_t = cst.tile([1, 128], F32R)
                nc.gpsimd.memset(ones_t[:], 1.0)
            if use_b1:
                b1r_t = cst.tile([EPC, F2], F32R)
                nc.gpsimd.dma_start(b1r_t[:], b1r[:])
            if use_b2:
                b2r_t = cst.tile([EPC, D], F32R)
                nc.gpsimd.dma_start(b2r_t[:], b2r[:])
            if use_bs1:
                b1s_t = cst.tile([1, F2], F32R)
                nc.gpsimd.dma_start(b1s_t[:], b1s[:])

            def swiglu_transpose(psum_h, tagsuffix):
                """psum_h [128,768] -> actT [128,3,128] (f-major, transposed)."""
                silu_t = spool.tile([128, F], F32, tag="silu")
                nc.scalar.activation(silu_t[:], psum_h[:, 0:F], AF.Silu)
                act_t = spool.tile([128, F], F32R, tag="act")
                nc.vector.tensor_mul(act_t[:], silu_t[:], psum_h[:, F:F2])
                actT = spool.tile([128, FT, 128], F32R, tag="actT" + tagsuffix)
                for ft in range(FT):
                    ptile = pt_pool.tile([128, 128], F32R)
                    nc.tensor.transpose(
                        ptile[:], act_t[:, ft * 128:(ft + 1) * 128], ident_t[:]
                    )
                    nc.vector.tensor_copy(actT[:, ft, :], ptile[:])
                return actT

            # ================= routed experts =================
            for e in range(EPC):
                psum_h = ph_pool.tile([128, F2], F32, tag="ph")
                if use_b1:
                    nc.tensor.matmul(psum_h[:, 0:512], ones_t[:, 0:CAP],
                                     b1r_t[e:e + 1, 0:512], start=True, stop=False)
                    nc.tensor.matmul(psum_h[:, 512:F2], ones_t[:, 0:CAP],
                                     b1r_t[e:e + 1, 512:F2], start=True, stop=False)
                for dtg in range(DTG):
                    xg_t = xpool.tile([128, 4, CAP], F32R, tag="xg")
                    nc.gpsimd.dma_start(xg_t[:], xg[e, dtg])
                    w1_t = wpool.tile([128, 4, F2], F32R, tag="w1")
                    nc.sync.dma_start(w1_t[:], w1p[e, dtg])
                    first = (dtg == 0) and not use_b1
                    for g in range(4):
                        nc.tensor.matmul(psum_h[:, 0:512], xg_t[:, g, :],
                                         w1_t[:, g, 0:512],
                                         start=first and g == 0, stop=False)
                        nc.tensor.matmul(psum_h[:, 512:F2], xg_t[:, g, :],
                                         w1_t[:, g, 512:F2],
                                         start=first and g == 0,
                                         stop=(dtg == DTG - 1 and g == 3))
                actT = swiglu_transpose(psum_h, "")
                ob = opool.tile([128, D], F32, tag="ob")
                for half in range(2):
                    w2_t = [None] * FT
                    for ft in range(FT):
                        w2_t[ft] = w2pool.tile([128, 2560], F32R, tag="w2")
                        nc.sync.dma_start(w2_t[ft][:], w2p[e, ft, half])
                    for ch in range(5):
                        po = po_pool.tile([128, 512], F32)
                        if use_b2:
                            nc.tensor.matmul(
                                po[:], ones_t[:, 0:CAP],
                                b2r_t[e:e + 1,
                                      half * 2560 + ch * 512:
                                      half * 2560 + (ch + 1) * 512],
                                start=True, stop=False)
                        for ft in range(FT):
                            nc.tensor.matmul(
                                po[:], actT[:, ft, :],
                                w2_t[ft][:, ch * 512:(ch + 1) * 512],
                                start=(ft == 0) and not use_b2,
                                stop=(ft == FT - 1))
                        nc.scalar.activation(
                            ob[:, half * 2560 + ch * 512:half * 2560 + (ch + 1) * 512],
                            po[:], AF.Copy, scale=cw_t[:, e:e + 1])
                nc.gpsimd.dma_start(out_r[e], ob[:])

            # ================= shared expert (this core's slice) ============
            psum_s = [ph_pool.tile([128, F2], F32, tag="ph") for _ in range(2)]
            if use_bs1:
                for tt in range(2):
                    nc.tensor.matmul(psum_s[tt][:, 0:512], ones_t[:, 0:128],
                                     b1s_t[:, 0:512], start=True, stop=False)
                    nc.tensor.matmul(psum_s[tt][:, 512:F2], ones_t[:, 0:128],
                                     b1s_t[:, 512:F2], start=True, stop=False)
            for dtg in range(DTG):
                xq_t = xpool.tile([128, 4, QT], F32R, tag="xq")
                nc.gpsimd.dma_start(xq_t[:], xq[dtg])
                w1s_t = wpool.tile([128, 4, F2], F32R, tag="w1")
                nc.sync.dma_start(w1s_t[:], w1sp[dtg])
                first = (dtg == 0) and not use_bs1
                for g in range(4):
                    for tt in range(2):
                        nc.tensor.matmul(
                            psum_s[tt][:, 0:512],
                            xq_t[:, g, tt * 128:(tt + 1) * 128],
                            w1s_t[:, g, 0:512],
                            start=first and g == 0, stop=False)
                        nc.tensor.matmul(
                            psum_s[tt][:, 512:F2],
                            xq_t[:, g, tt * 128:(tt + 1) * 128],
                            w1s_t[:, g, 512:F2],
                            start=first and g == 0,
                            stop=(dtg == DTG - 1 and g == 3))
            actTs = [swiglu_transpose(psum_s[tt], "s") for tt in range(2)]
            obs = [opool.tile([128, D], F32, tag="ob") for _ in range(2)]
            for half in range(2):
                w2s_t = [None] * FT
                for ft in range(FT):
                    w2s_t[ft] = w2pool.tile([128, 2560], F32R, tag="w2")
                    nc.sync.dma_start(w2s_t[ft][:], w2sp[ft, half])
                for tt in range(2):
                    for ch in range(5):
                        po = po_pool.tile([128, 512], F32)
                        for ft in range(FT):
                            nc.tensor.matmul(
                                po[:], actTs[tt][:, ft, :],
                                w2s_t[ft][:, ch * 512:(ch + 1) * 512],
                                start=(ft == 0), stop=(ft == FT - 1))
                        nc.vector.tensor_copy(
                            obs[tt][:, half * 2560 + ch * 512:
                                    half * 2560 + (ch + 1) * 512], po[:])
            for tt in range(2):
                nc.gpsimd.dma_start(out_s[tt * 128:(tt + 1) * 128, :], obs[tt][:])

    nc.compile()
    return nc


def _get_nc(key):
    if key not in _compiled:
        _compiled[key] = _build(*key)
    return _compiled[key]


def _silu(v):
    return v / (1.0 + np.exp(-v))


def _pack_w1(w):  # [D, 2F] -> [DTG, 128, 4, 2F]
    return np.ascontiguousarray(
        w.reshape(DTG, 4, 128, F2).transpose(0, 2, 1, 3))


def _pack_w2(w):  # [F, D] -> [FT, 2, 128, 2560]
    return np.ascontiguousarray(
        w.reshape(FT, 128, 2, 2560).transpose(0, 2, 1, 3))


def _pack_xT(xt_cols):  # [D, ncols] -> [DTG, 128, 4, ncols]
    n = xt_cols.shape[1]
    return np.ascontiguousarray(
        xt_cols.reshape(DTG, 4, 128, n).transpose(0, 2, 1, 3))


def kernel(x, gate_w, gate_b, shared_w1, shared_b1, shared_w2, shared_b2,
           routed_w1, routed_b1, routed_w2, routed_b2):
    from concourse.bass_utils import run_bass_kernel_spmd

    f32 = np.float32
    x = np.asarray(x, f32)
    gate_w = np.asarray(gate_w, f32)
    gate_b = np.asarray(gate_b, f32)
    shared_w1 = np.asarray(shared_w1, f32)
    shared_b1 = np.asarray(shared_b1, f32)
    shared_w2 = np.asarray(shared_w2, f32)
    shared_b2 = np.asarray(shared_b2, f32)
    routed_w1 = np.asarray(routed_w1, f32)
    routed_b1 = np.asarray(routed_b1, f32)
    routed_w2 = np.asarray(routed_w2, f32)
    routed_b2 = np.asarray(routed_b2, f32)

    B = x.shape[0]
    x2 = x.reshape(T, D)

    # ---- gate: softmax + top-2 (unnormalized combine weights) ----
    logits = x2 @ gate_w + gate_b
    m = logits.max(-1, keepdims=True)
    p = np.exp(logits - m, dtype=f32)
    p = p / p.sum(-1, keepdims=True)
    ar = np.arange(T)
    i1 = np.argmax(p, -1)
    p1 = p[ar, i1]
    pm = p.copy()
    pm[ar, i1] = -1.0
    i2 = np.argmax(pm, -1)
    p2 = p[ar, i2]

    # per-expert token lists (stable order)
    pairs = np.concatenate([i1, i2])
    toks = np.concatenate([ar, ar])
    wts = np.concatenate([p1, p2]).astype(f32)
    order = np.argsort(pairs, kind="stable")
    pairs_s, toks_s, wts_s = pairs[order], toks[order], wts[order]
    counts = np.bincount(pairs, minlength=E)
    starts = np.zeros(E + 1, np.int64)
    np.cumsum(counts, out=starts[1:])

    sel_tok = [None] * E
    sel_wt = [None] * E
    overflow = []
    for e in range(E):
        te = toks_s[starts[e]:starts[e + 1]]
        we = wts_s[starts[e]:starts[e + 1]]
        if len(te) > CAP:
            overflow.append((e, te[CAP:], we[CAP:]))
            te, we = te[:CAP], we[:CAP]
        sel_tok[e] = te
        sel_wt[e] = we

    use_b1 = bool(np.any(routed_b1))
    use_b2 = bool(np.any(routed_b2))
    use_bs1 = bool(np.any(shared_b1))
    nc = _get_nc((use_b1, use_b2, use_bs1, KDT))

    kdt = _np_kdt()
    ident_np = np.eye(128, dtype=kdt)
    xT = np.ascontiguousarray(x2.T).astype(kdt)  # [D, T]
    routed_w1k = routed_w1.astype(kdt)
    routed_w2k = routed_w2.astype(kdt)
    shared_w1k = shared_w1.astype(kdt)
    shared_w2k = shared_w2.astype(kdt)

    in_maps = []
    for c in range(NCORE):
        es = [4 * c + i for i in range(EPC)]
        # gathered-padded tokens, one 128-slot per expert: [D, 4*CAP]
        idx_pad = np.zeros(EPC * CAP, np.int64)
        cw_pad = np.zeros((CAP, EPC), f32)
        for i, e in enumerate(es):
            n = len(sel_tok[e])
            idx_pad[i * CAP:i * CAP + n] = sel_tok[e]
            cw_pad[:n, i] = sel_wt[e]
        xg_cols = xT[:, idx_pad]  # [D, 512]
        xg_np = np.stack([
            _pack_xT(xg_cols[:, i * CAP:(i + 1) * CAP]) for i in range(EPC)])
        w1p_np = np.stack([_pack_w1(routed_w1k[e]) for e in es])
        w2p_np = np.stack([_pack_w2(routed_w2k[e]) for e in es])

        s_c, q_c = c % S, c // S
        xq_np = _pack_xT(xT[:, q_c * QT:(q_c + 1) * QT])
        w1sp_np = _pack_w1(shared_w1k[s_c])
        w2sp_np = _pack_w2(shared_w2k[s_c])

        im = {
            "xg": xg_np, "w1p": w1p_np, "w2p": w2p_np, "cwc": cw_pad,
            "xq": xq_np, "w1sp": w1sp_np, "w2sp": w2sp_np, "ident": ident_np,
        }
        if use_b1:
            im["b1r"] = np.ascontiguousarray(routed_b1[es]).astype(kdt)
        if use_b2:
            im["b2r"] = np.ascontiguousarray(routed_b2[es]).astype(kdt)
        if use_bs1:
            im["b1s"] = shared_b1[s_c:s_c + 1].astype(kdt)
        in_maps.append(im)

    res = run_bass_kernel_spmd(nc, in_maps, core_ids=list(range(NCORE)))

    # ---- host gather/unshard ----
    # routed: each valid (expert, slot) row is c_t * expert_out(token)
    R = np.concatenate([res.results[c]["out_r"] for c in range(NCORE)], axis=0)
    R = R.reshape(E * CAP, D)
    tok_of_row = np.full(E * CAP, -1, np.int64)
    valid = np.zeros(E * CAP, bool)
    for e in range(E):
        n = len(sel_tok[e])
        tok_of_row[e * CAP:e * CAP + n] = sel_tok[e]
        valid[e * CAP:e * CAP + n] = True
    vrows = np.flatnonzero(valid)
    tv = tok_of_row[vrows]
    o = np.argsort(tv, kind="stable")
    out = np.zeros((T, D), f32)
    n_entries = np.bincount(tv, minlength=T)
    if n_entries.max() <= 2 and not overflow and n_entries.min() == 2:
        rows_sorted = vrows[o]
        out += R[rows_sorted[0::2]]
        out += R[rows_sorted[1::2]]
    else:
        np.add.at(out, tv, R[vrows])
    # overflow tokens: exact host fallback
    for e, te, we in overflow:
        xv = x2[te]
        h = xv @ routed_w1[e] + routed_b1[e]
        act = _silu(h[:, :F]) * h[:, F:]
        out[te] += we[:, None] * (act @ routed_w2[e] + routed_b2[e])

    # shared: quarters q handled by cores 2q (expert 0) and 2q+1 (expert 1)
    for q in range(NCORE // S):
        out[q * QT:(q + 1) * QT] += res.results[S * q]["out_s"]
        out[q * QT:(q + 1) * QT] += res.results[S * q + 1]["out_s"]
    out += shared_b2.sum(0)[None, :]

    return out.reshape(B, T, D).astype(f32)


# revision 5
# speedup vs baseline: 1.9329x; 1.2551x over previous
"""DeepSeekMoE Trainium2 kernel: 8-core expert-parallel sparse dispatch.

Strategy (hardcoded for D=5120, F=384, E=32, S=2, T=1024, top-2):
- Host computes the gate (softmax + top-2 + combine weights) and dispatches
  tokens: each of the 8 cores owns 4 routed experts; its assigned tokens are
  gathered, transposed and padded to 128 per expert on the host.
- Each core runs a Bass/Tile kernel: per expert a [tokens x D] @ [D x 2F]
  fc1 (tokens stationary, weights moving, fp32r), SwiGLU, PE transpose,
  fc2, and per-token combine scaling fused into the PSUM->SBUF copy.
- Shared experts are sharded expert x token-quarter (core c handles shared
  expert c%2 for tokens [256*(c//2), 256*(c//2+1))), so shared weights are
  not replicated across all cores.
- Host gathers: routed outputs are scatter-added via two vectorized gathers
  (each token has exactly 2 expert contributions), shared quarters are
  summed pairwise.
"""
import sys
import os

sys.path.insert(0, "/opt/trn_rl_repo")

import numpy as np

D = 5120
F = 384
F2 = 768
E = 32
S = 2
T = 1024
NCORE = 8
EPC = E // NCORE          # experts per core
CAP = 128                 # token capacity per expert slot
QT = T // (NCORE // S)    # tokens per shared quarter = 256
DT = D // 128             # 40 d-tiles
DTG = DT // 4             # 10 groups of 4 d-tiles
FT = F // 128             # 3 f-tiles

KDT = "bf16"   # matmul compute dtype: "bf16" or "f32r"
_compiled = {}


def _np_kdt():
    import concourse.mybir as mybir
    return mybir.dt.np(mybir.dt.bfloat16 if KDT == "bf16" else mybir.dt.float32r)


def _build(use_b1, use_b2, use_bs1, kdt):
    import concourse.bass as bass
    import concourse.bacc as bacc
    import concourse.tile as tile
    import concourse.mybir as mybir

    F32 = mybir.dt.float32
    F32R = mybir.dt.bfloat16 if kdt == "bf16" else mybir.dt.float32r
    AF = mybir.ActivationFunctionType

    nc = bacc.Bacc(None, target_bir_lowering=False)

    # ---- DRAM I/O ----
    # routed
    xg = nc.dram_tensor("xg", [EPC, DTG, 128, 4, CAP], F32R, kind="ExternalInput")
    w1p = nc.dram_tensor("w1p", [EPC, DTG, 128, 4, F2], F32R, kind="ExternalInput")
    w2p = nc.dram_tensor("w2p", [EPC, FT, 2, 128, 2560], F32R, kind="ExternalInput")
    cwc = nc.dram_tensor("cwc", [CAP, EPC], F32, kind="ExternalInput")
    out_r = nc.dram_tensor("out_r", [EPC, CAP, D], F32, kind="ExternalOutput")
    # shared (this core's expert s=c%2, token quarter q=c//2)
    xq = nc.dram_tensor("xq", [DTG, 128, 4, QT], F32R, kind="ExternalInput")
    w1sp = nc.dram_tensor("w1sp", [DTG, 128, 4, F2], F32R, kind="ExternalInput")
    w2sp = nc.dram_tensor("w2sp", [FT, 2, 128, 2560], F32R, kind="ExternalInput")
    out_s = nc.dram_tensor("out_s", [QT, D], F32, kind="ExternalOutput")
    # constants
    ident = nc.dram_tensor("ident", [128, 128], F32R, kind="ExternalInput")
    if use_b1:
        b1r = nc.dram_tensor("b1r", [EPC, F2], F32R, kind="ExternalInput")
    if use_b2:
        b2r = nc.dram_tensor("b2r", [EPC, D], F32R, kind="ExternalInput")
    if use_bs1:
        b1s = nc.dram_tensor("b1s", [1, F2], F32R, kind="ExternalInput")

    with tile.TileContext(nc) as tc:
        with (
            tc.tile_pool(name="cst", bufs=1) as cst,
            tc.tile_pool(name="wpool", bufs=6) as wpool,
            tc.tile_pool(name="w2pool", bufs=6) as w2pool,
            tc.tile_pool(name="xpool", bufs=6) as xpool,
            tc.tile_pool(name="spool", bufs=2) as spool,
            tc.tile_pool(name="opool", bufs=2) as opool,
            tc.tile_pool(name="ph", bufs=2, space="PSUM") as ph_pool,
            tc.tile_pool(name="pt", bufs=2, space="PSUM") as pt_pool,
            tc.tile_pool(name="po", bufs=2, space="PSUM") as po_pool,
        ):
            ident_t = cst.tile([128, 128], F32R)
            nc.gpsimd.dma_start(ident_t[:], ident[:])
            cw_t = cst.tile([CAP, EPC], F32)
            nc.gpsimd.dma_start(cw_t[:], cwc[:])
            need_ones = use_b1 or use_b2 or use_bs1
            if need_ones:
                ones_t = cst.tile([1, 128], F32R)
                nc.gpsimd.memset(ones_t[:], 1.0)
            if use_b1:
                b1r_t = cst.tile([EPC, F2], F32R)
                nc.gpsimd.dma_start(b1r_t[:], b1r[:])
            if use_b2:
                b2r_t = cst.tile([EPC, D], F32R)
                nc.gpsimd.dma_start(b2r_t[:], b2r[:])
            if use_bs1:
                b1s_t = cst.tile([1, F2], F32R)
                nc.gpsimd.dma_start(b1s_t[:], b1s[:])

            def swiglu_transpose(psum_h, tagsuffix):
                """psum_h [128,768] -> actT [128,3,128] (f-major, transposed)."""
                silu_t = spool.tile([128, F], F32, tag="silu")
                nc.scalar.activation(silu_t[:], psum_h[:, 0:F], AF.Silu)
                act_t = spool.tile([128, F], F32R, tag="act")
                nc.vector.tensor_mul(act_t[:], silu_t[:], psum_h[:, F:F2])
                actT = spool.tile([128, FT, 128], F32R, tag="actT" + tagsuffix)
                for ft in range(FT):
                    ptile = pt_pool.tile([128, 128], F32R)
                    nc.tensor.transpose(
                        ptile[:], act_t[:, ft * 128:(ft + 1) * 128], ident_t[:]
                    )
                    nc.vector.tensor_copy(actT[:, ft, :], ptile[:])
                return actT

            # ================= routed experts =================
            for e in range(EPC):
                psum_h = ph_pool.tile([128, F2], F32, tag="ph")
                if use_b1:
                    nc.tensor.matmul(psum_h[:, 0:512], ones_t[:, 0:CAP],
                                     b1r_t[e:e + 1, 0:512], start=True, stop=False)
                    nc.tensor.matmul(psum_h[:, 512:F2], ones_t[:, 0:CAP],
                                     b1r_t[e:e + 1, 512:F2], start=True, stop=False)
                for dtg in range(DTG):
                    xg_t = xpool.tile([128, 4, CAP], F32R, tag="xg")
                    nc.gpsimd.dma_start(xg_t[:], xg[e, dtg])
                    w1_t = wpool.tile([128, 4, F2], F32R, tag="w1")
                    nc.sync.dma_start(w1_t[:], w1p[e, dtg])
                    first = (dtg == 0) and not use_b1
                    for g in range(4):
                        nc.tensor.matmul(psum_h[:, 0:512], xg_t[:, g, :],
                                         w1_t[:, g, 0:512],
                                         start=first and g == 0, stop=False)
                        nc.tensor.matmul(psum_h[:, 512:F2], xg_t[:, g, :],
                                         w1_t[:, g, 512:F2],
                                         start=first and g == 0,
                                         stop=(dtg == DTG - 1 and g == 3))
                actT = swiglu_transpose(psum_h, "")
                ob = opool.tile([128, D], F32, tag="ob")
                for half in range(2):
                    w2_t = [None] * FT
                    for ft in range(FT):
                        w2_t[ft] = w2pool.tile([128, 2560], F32R, tag="w2")
                        nc.sync.dma_start(w2_t[ft][:], w2p[e, ft, half])
                    for ch in range(5):
                        po = po_pool.tile([128, 512], F32)
                        if use_b2:
                            nc.tensor.matmul(
                                po[:], ones_t[:, 0:CAP],
                                b2r_t[e:e + 1,
                                      half * 2560 + ch * 512:
                                      half * 2560 + (ch + 1) * 512],
                                start=True, stop=False)
                        for ft in range(FT):
                            nc.tensor.matmul(
                                po[:], actT[:, ft, :],
                                w2_t[ft][:, ch * 512:(ch + 1) * 512],
                                start=(ft == 0) and not use_b2,
                                stop=(ft == FT - 1))
                        nc.scalar.activation(
                            ob[:, half * 2560 + ch * 512:half * 2560 + (ch + 1) * 512],
                            po[:], AF.Copy, scale=cw_t[:, e:e + 1])
                nc.gpsimd.dma_start(out_r[e], ob[:])

            # ================= shared expert (this core's slice) ============
            psum_s = [ph_pool.tile([128, F2], F32, tag="ph") for _ in range(2)]
            if use_bs1:
                for tt in range(2):
                    nc.tensor.matmul(psum_s[tt][:, 0:512], ones_t[:, 0:128],
                                     b1s_t[:, 0:512], start=True, stop=False)
                    nc.tensor.matmul(psum_s[tt][:, 512:F2], ones_t[:, 0:128],
                                     b1s_t[:, 512:F2], start=True, stop=False)
            for dtg in range(DTG):
                xq_t = xpool.tile([128, 4, QT], F32R, tag="xq")
                nc.gpsimd.dma_start(xq_t[:], xq[dtg])
                w1s_t = wpool.tile([128, 4, F2], F32R, tag="w1")
                nc.sync.dma_start(w1s_t[:], w1sp[dtg])
                first = (dtg == 0) and not use_bs1
                for g in range(4):
                    for tt in range(2):
                        nc.tensor.matmul(
                            psum_s[tt][:, 0:512],
                            xq_t[:, g, tt * 128:(tt + 1) * 128],
                            w1s_t[:, g, 0:512],
                            start=first and g == 0, stop=False)
                        nc.tensor.matmul(
                            psum_s[tt][:, 512:F2],
                            xq_t[:, g, tt * 128:(tt + 1) * 128],
                            w1s_t[:, g, 512:F2],
                            start=first and g == 0,
                            stop=(dtg == DTG - 1 and g == 3))
            actTs = [swiglu_transpose(psum_s[tt], "s") for tt in range(2)]
            obs = [opool.tile([128, D], F32, tag="ob") for _ in range(2)]
            for half in range(2):
                w2s_t = [None] * FT
                for ft in range(FT):
                    w2s_t[ft] = w2pool.tile([128, 2560], F32R, tag="w2")
                    nc.sync.dma_start(w2s_t[ft][:], w2sp[ft, half])
                for tt in range(2):
                    for ch in range(5):
                        po = po_pool.tile([128, 512], F32)
                        for ft in range(FT):
                            nc.tensor.matmul(
                                po[:], actTs[tt][:, ft, :],
                                w2s_t[ft][:, ch * 512:(ch + 1) * 512],
                                start=(ft == 0), stop=(ft == FT - 1))
                        nc.vector.tensor_copy(
                            obs[tt][:, half * 2560 + ch * 512:
                                    half * 2560 + (ch + 1) * 512], po[:])
            for tt in range(2):
                nc.gpsimd.dma_start(out_s[tt * 128:(tt + 1) * 128, :], obs[tt][:])

    nc.compile()
    return nc


def _get_nc(key):
    if key not in _compiled:
        _compiled[key] = _build(*key)
    return _compiled[key]


def _silu(v):
    return v / (1.0 + np.exp(-v))


def _pack_w1(w):  # [D, 2F] -> [DTG, 128, 4, 2F]
    return np.ascontiguousarray(
        w.reshape(DTG, 4, 128, F2).transpose(0, 2, 1, 3))


def _pack_w2(w):  # [F, D] -> [FT, 2, 128, 2560]
    return np.ascontiguousarray(
        w.reshape(FT, 128, 2, 2560).transpose(0, 2, 1, 3))


def _pack_xT(xt_cols):  # [D, ncols] -> [DTG, 128, 4, ncols]
    n = xt_cols.shape[1]
    return np.ascontiguousarray(
        xt_cols.reshape(DTG, 4, 128, n).transpose(0, 2, 1, 3))


def kernel(x, gate_w, gate_b, shared_w1, shared_b1, shared_w2, shared_b2,
           routed_w1, routed_b1, routed_w2, routed_b2):
    from concourse.bass_utils import run_bass_kernel_spmd

    f32 = np.float32
    x = np.asarray(x, f32)
    gate_w = np.asarray(gate_w, f32)
    gate_b = np.asarray(gate_b, f32)
    shared_w1 = np.asarray(shared_w1, f32)
    shared_b1 = np.asarray(shared_b1, f32)
    shared_w2 = np.asarray(shared_w2, f32)
    shared_b2 = np.asarray(shared_b2, f32)
    routed_w1 = np.asarray(routed_w1, f32)
    routed_b1 = np.asarray(routed_b1, f32)
    routed_w2 = np.asarray(routed_w2, f32)
    routed_b2 = np.asarray(routed_b2, f32)

    B = x.shape[0]
    x2 = x.reshape(T, D)

    # ---- gate: softmax + top-2 (unnormalized combine weights) ----
    logits = x2 @ gate_w + gate_b
    m = logits.max(-1, keepdims=True)
    p = np.exp(logits - m, dtype=f32)
    p = p / p.sum(-1, keepdims=True)
    ar = np.arange(T)
    i1 = np.argmax(p, -1)
    p1 = p[ar, i1]
    pm = p.copy()
    pm[ar, i1] = -1.0
    i2 = np.argmax(pm, -1)
    p2 = p[ar, i2]

    # per-expert token lists (stable order)
    pairs = np.concatenate([i1, i2])
    toks = np.concatenate([ar, ar])
    wts = np.concatenate([p1, p2]).astype(f32)
    order = np.argsort(pairs, kind="stable")
    pairs_s, toks_s, wts_s = pairs[order], toks[order], wts[order]
    counts = np.bincount(pairs, minlength=E)
    starts = np.zeros(E + 1, np.int64)
    np.cumsum(counts, out=starts[1:])

    sel_tok = [None] * E
    sel_wt = [None] * E
    overflow = []
    for e in range(E):
        te = toks_s[starts[e]:starts[e + 1]]
        we = wts_s[starts[e]:starts[e + 1]]
        if len(te) > CAP:
            overflow.append((e, te[CAP:], we[CAP:]))
            te, we = te[:CAP], we[:CAP]
        sel_tok[e] = te
        sel_wt[e] = we

    use_b1 = bool(np.any(routed_b1))
    use_b2 = bool(np.any(routed_b2))
    use_bs1 = bool(np.any(shared_b1))
    nc = _get_nc((use_b1, use_b2, use_bs1, KDT))

    kdt = _np_kdt()
    ident_np = np.eye(128, dtype=kdt)
    xT = np.ascontiguousarray(x2.T).astype(kdt)  # [D, T]
    routed_w1k = routed_w1.astype(kdt)
    routed_w2k = routed_w2.astype(kdt)
    shared_w1k = shared_w1.astype(kdt)
    shared_w2k = shared_w2.astype(kdt)

    in_maps = []
    for c in range(NCORE):
        es = [4 * c + i for i in range(EPC)]
        # gathered-padded tokens, one 128-slot per expert: [D, 4*CAP]
        idx_pad = np.zeros(EPC * CAP, np.int64)
        cw_pad = np.zeros((CAP, EPC), f32)
        for i, e in enumerate(es):
            n = len(sel_tok[e])
            idx_pad[i * CAP:i * CAP + n] = sel_tok[e]
            cw_pad[:n, i] = sel_wt[e]
        xg_cols = xT[:, idx_pad]  # [D, 512]
        xg_np = np.stack([
            _pack_xT(xg_cols[:, i * CAP:(i + 1) * CAP]) for i in range(EPC)])
        w1p_np = np.stack([_pack_w1(routed_w1k[e]) for e in es])
        w2p_np = np.stack([_pack_w2(routed_w2k[e]) for e in es])

        s_c, q_c = c % S, c // S
        xq_np = _pack_xT(xT[:, q_c * QT:(q_c + 1) * QT])
        w1sp_np = _pack_w1(shared_w1k[s_c])
        w2sp_np = _pack_w2(shared_w2k[s_c])

        im = {
            "xg": xg_np, "w1p": w1p_np, "w2p": w2p_np, "cwc": cw_pad,
            "xq": xq_np, "w1sp": w1sp_np, "w2sp": w2sp_np, "ident": ident_np,
        }
        if use_b1:
            im["b1r"] = np.ascontiguousarray(routed_b1[es]).astype(kdt)
        if use_b2:
            im["b2r"] = np.ascontiguousarray(routed_b2[es]).astype(kdt)
        if use_bs1:
            im["b1s"] = shared_b1[s_c:s_c + 1].astype(kdt)
        in_maps.append(im)

    res = run_bass_kernel_spmd(nc, in_maps, core_ids=list(range(NCORE)))

    # ---- host gather/unshard ----
    # routed: each valid (expert, slot) row is c_t * expert_out(token)
    R = np.concatenate([np.asarray(res.results[c]["out_r"], np.float32) for c in range(NCORE)], axis=0)
    R = R.reshape(E * CAP, D)
    tok_of_row = np.full(E * CAP, -1, np.int64)
    valid = np.zeros(E * CAP, bool)
    for e in range(E):
        n = len(sel_tok[e])
        tok_of_row[e * CAP:e * CAP + n] = sel_tok[e]
        valid[e * CAP:e * CAP + n] = True
    vrows = np.flatnonzero(valid)
    tv = tok_of_row[vrows]
    o = np.argsort(tv, kind="stable")
    out = np.zeros((T, D), f32)
    n_entries = np.bincount(tv, minlength=T)
    if n_entries.max() <= 2 and not overflow and n_entries.min() == 2:
        rows_sorted = vrows[o]
        out += R[rows_sorted[0::2]]
        out += R[rows_sorted[1::2]]
    else:
        np.add.at(out, tv, R[vrows])
    # overflow tokens: exact host fallback
    for e, te, we in overflow:
        xv = x2[te]
        h = xv @ routed_w1[e] + routed_b1[e]
        act = _silu(h[:, :F]) * h[:, F:]
        out[te] += we[:, None] * (act @ routed_w2[e] + routed_b2[e])

    # shared: quarters q handled by cores 2q (expert 0) and 2q+1 (expert 1)
    for q in range(NCORE // S):
        out[q * QT:(q + 1) * QT] += np.asarray(res.results[S * q]["out_s"], np.float32)
        out[q * QT:(q + 1) * QT] += np.asarray(res.results[S * q + 1]["out_s"], np.float32)
    out += shared_b2.sum(0)[None, :]

    return out.reshape(B, T, D).astype(f32)
